# revision 5
# baseline (speedup 1.0000x reference)
"""Trainium2 Bass kernel for NeuralSparseHG deterministic top-k masking.

Strategy (sharding_hint): logits are sharded row-wise (by node) across the
8 cores; edge incidences are routed (grouped) to the core/partition owning
their V_idx, so the segmented top-k is fully local per shard.

Device work per core (nodes are 1024 contiguous rows; 8 tiles of 128 rows):
  - stream the 128-row logits tile into SBUF (the memory-bound term),
  - GPSIMD indirect_copy gathers, for each 16-node group, the group's
    (padded) incident-edge columns; non-owner partitions are masked to
    -1e30 with a host-built owner mask, pad slots hit a -1e30 pad column,
  - vector.max gives each node's top-8 values; the 5th largest is the
    threshold T; keep = (x > T) or (x == T and tie-rank < 5 - #gt),
    with tie-rank via a prefix-sum scan (matches the reference lexsort's
    stable ordering, including duplicate (v,e) incidences),
  - ACT computes sigmoid scores; the keep bit is encoded in the sign of
    the emitted per-slot score: out = (keep - 0.5) * sigmoid(logit).

Host only shards/routes inputs and inverse-permutes the outputs.
"""

import numpy as np

NUM_NODES = 10000
NUM_USED_NODES = 8192  # V_idx is drawn from [0, 8192)
NUM_EDGES = 8192
NNZ = 320000
TOP_K = 5

N_CORES = 8
NODES_PER_CORE = 1024
TILES_PER_CORE = 8
ROWS_PER_TILE = 128
C = 736            # flat slots per 16-node group (observed max 697)
IDXW = C // 16     # wrapped index columns (uint16)
PAD_COL = 0        # pad slots gather column 0; owner mask hides them
DATA_W = 8192

_CACHE = {}


def _build(reps: int = 1):
    """Trace + compile the SPMD kernel; returns a reusable jitted runner."""
    key = ("runner", reps)
    if key in _CACHE:
        return _CACHE[key]

    import jax
    from jax.sharding import Mesh, PartitionSpec, NamedSharding
    from jax.experimental.shard_map import shard_map
    import concourse.bacc as bacc
    import concourse.mybir as mybir
    from concourse.tile import TileContext
    from concourse.bass2jax import (
        _bass_exec_p,
        install_neuronx_cc_hook,
        partition_id_tensor,
    )

    nc = bacc.Bacc("TRN2", target_bir_lowering=False, debug=False,
                   num_devices=N_CORES)
    f32, u8, u16 = mybir.dt.float32, mybir.dt.uint8, mybir.dt.uint16
    T, P = TILES_PER_CORE, ROWS_PER_TILE
    logits_d = nc.dram_tensor("logits", [NODES_PER_CORE, NUM_EDGES], f32,
                              kind="ExternalInput")
    idx_d = nc.dram_tensor("idx", [T, P, IDXW], u16, kind="ExternalInput")
    v8_d = nc.dram_tensor("v8", [T, P, C], u8, kind="ExternalInput")
    out_d = nc.dram_tensor("out", [T, P, C], f32, kind="ExternalOutput")

    Alu = mybir.AluOpType
    with TileContext(nc) as tc:
        # The indirect_copy ucode's 3-index read pattern issues a 4th
        # (dropped) read at element offset idx1+idx2-idx0, which can fall
        # up to ~32KB below/above the data tile. The guard tile keeps those
        # addresses inside valid SBUF.
        with tc.tile_pool(name="gpool", bufs=1) as gpool, \
             tc.tile_pool(name="zpool", bufs=1) as zpool, \
             tc.tile_pool(name="dpool", bufs=3) as dpool, \
             tc.tile_pool(name="spool", bufs=2) as spool:
            guard = gpool.tile([P, 8448], f32, tag="guard")  # noqa: F841
            zers = zpool.tile([P, C], f32, tag="zeros")
            nc.vector.memset(zers[:], 0.0)
            for _ in range(reps):
                for t in range(T):
                    data = dpool.tile([P, DATA_W], f32, tag="data")
                    idx = spool.tile([P, IDXW], u16, tag="idx")
                    v8 = spool.tile([P, C], u8, tag="v8")
                    gath = spool.tile([P, C], f32, tag="gath")
                    mask = spool.tile([P, C], f32, tag="mask")
                    mx = spool.tile([P, 8], f32, tag="mx")
                    tpp = spool.tile([P, 1], f32, tag="tpp")
                    gt = spool.tile([P, C], f32, tag="gt")
                    cgt = spool.tile([P, 1], f32, tag="cgt")
                    eq = spool.tile([P, C], f32, tag="eq")
                    rr = spool.tile([P, C], f32, tag="rr")
                    keq = spool.tile([P, C], f32, tag="keq")
                    kp = spool.tile([P, C], f32, tag="kp")
                    sg = spool.tile([P, C], f32, tag="sg")
                    sgn = spool.tile([P, C], f32, tag="sgn")

                    nc.sync.dma_start(out=data[:], in_=logits_d[t * P:(t + 1) * P, :])
                    nc.sync.dma_start(out=idx[:], in_=idx_d[t])
                    nc.sync.dma_start(out=v8[:], in_=v8_d[t])

                    nc.gpsimd.indirect_copy(gath[:], data[:], idx[:],
                                            i_know_ap_gather_is_preferred=True)
                    # masked = gathered + owner_mask * -1e30
                    nc.vector.scalar_tensor_tensor(mask[:], v8[:], -1e30, gath[:],
                                                   op0=Alu.mult, op1=Alu.add)
                    # per-node top-8 (multiset, descending); T = mx[:, 4]
                    nc.vector.max(mx[:], mask[:])
                    # tpp = T, or +1e30 when T == -1e30 (fewer than 5 real slots)
                    nc.vector.tensor_scalar(tpp[:], mx[:, 4:5], -1e30, 2e30,
                                            op0=Alu.is_equal, op1=Alu.mult)
                    nc.vector.tensor_tensor(tpp[:], tpp[:], mx[:, 4:5], op=Alu.add)
                    # gt = masked > T ; cgt = row-sum(gt)
                    nc.vector.tensor_scalar(gt[:], mask[:], mx[:, 4:5], None,
                                            op0=Alu.is_gt, op1=Alu.add,
                                            accum_out=cgt[:])
                    # eq = masked == T'' (only real owned slots can match)
                    nc.vector.tensor_scalar(eq[:], mask[:], tpp[:, 0:1], None,
                                            op0=Alu.is_equal)
                    # rr = cgt + inclusive-cumsum(eq): rank+1 among kept-candidates
                    nc.vector.tensor_tensor_scan(rr[:], eq[:], zers[:], cgt[:, 0:1],
                                                 op0=Alu.add, op1=Alu.add)
                    # keep-at-threshold: eq and rank < 5  <=>  rr <= 5
                    nc.vector.scalar_tensor_tensor(keq[:], rr[:], 5.5, eq[:],
                                                   op0=Alu.is_lt, op1=Alu.mult)
                    nc.vector.tensor_tensor(kp[:], gt[:], keq[:], op=Alu.add)
                    # scores on ACT; signed = (keep - 0.5) * sigmoid
                    nc.scalar.activation(sg[:], mask[:],
                                         mybir.ActivationFunctionType.Sigmoid)
                    nc.vector.scalar_tensor_tensor(sgn[:], kp[:], 0.5, sg[:],
                                                   op0=Alu.subtract, op1=Alu.mult)
                    nc.sync.dma_start(out=out_d[t], in_=sgn[:])
    nc.compile()
    _CACHE[("nc", reps)] = nc

    install_neuronx_cc_hook()
    partition_name = nc.partition_id_tensor.name if nc.partition_id_tensor else None
    in_names, out_names, out_avals, zero_outs = [], [], [], []
    for alloc in nc.m.functions[0].allocations:
        if not isinstance(alloc, mybir.MemoryLocationSet):
            continue
        name = alloc.memorylocations[0].name
        if alloc.kind == "ExternalInput":
            if name != partition_name:
                in_names.append(name)
        elif alloc.kind == "ExternalOutput":
            out_names.append(name)
            shape = tuple(alloc.tensor_shape)
            dtype = mybir.dt.np(alloc.dtype)
            out_avals.append(jax.core.ShapedArray(shape, dtype))
            zero_outs.append(np.zeros(shape, dtype))
    all_names = in_names + out_names + ([partition_name] if partition_name else [])

    def _body(*args):
        operands = list(args)
        if partition_name is not None:
            operands.append(partition_id_tensor())
        outs = _bass_exec_p.bind(
            *operands,
            out_avals=tuple(out_avals),
            in_names=tuple(all_names),
            out_names=tuple(out_names),
            lowering_input_output_aliases=(),
            sim_require_finite=True,
            sim_require_nnan=True,
            nc=nc,
        )
        return tuple(outs)

    devices = jax.devices()[:N_CORES]
    mesh = Mesh(np.asarray(devices), ("core",))
    n_params = len(in_names)
    sharded = jax.jit(
        shard_map(_body, mesh=mesh,
                  in_specs=(PartitionSpec("core"),) * (n_params + len(out_names)),
                  out_specs=(PartitionSpec("core"),) * len(out_names),
                  check_rep=False),
        keep_unused=True,
    )

    class Runner:
        def __init__(self):
            self.in_names = in_names
            self.out_names = out_names
            self.out_avals = out_avals
            self.sharded = sharded
            self.mesh = mesh
            self._placed = None

        def place(self, in_maps):
            per_core = [[np.asarray(in_maps[c][n]) for n in in_names]
                        for c in range(N_CORES)]
            concat_in = [np.concatenate([per_core[c][i] for c in range(N_CORES)], axis=0)
                         for i in range(n_params)]
            concat_zero = [np.zeros((N_CORES * z.shape[0], *z.shape[1:]), z.dtype)
                           for z in zero_outs]
            sh = NamedSharding(mesh, PartitionSpec("core"))
            self._placed = [jax.device_put(a, sh) for a in (*concat_in, *concat_zero)]

        def run_placed(self):
            out = self.sharded(*self._placed)
            jax.block_until_ready(out)
            return out

        def run(self, in_maps):
            self.place(in_maps)
            out_arrs = self.run_placed()
            return [
                {n: np.asarray(out_arrs[i]).reshape(N_CORES, *out_avals[i].shape)[c]
                 for i, n in enumerate(out_names)}
                for c in range(N_CORES)
            ]

    runner = Runner()
    _CACHE[key] = runner
    return runner


def _pack(edge_index):
    """Group incidences by node; build per-core gather indices and owner masks."""
    V = np.asarray(edge_index[0]).astype(np.int64)
    E = np.asarray(edge_index[1]).astype(np.int64)
    order = np.argsort(V, kind="stable")
    Vs, Es = V[order], E[order]
    counts = np.bincount(Vs, minlength=NUM_USED_NODES)
    gcounts = counts.reshape(-1, 16).sum(1)
    gstart = np.concatenate([[0], np.cumsum(gcounts)[:-1]])
    g16 = Vs >> 4
    pos = np.arange(Vs.shape[0]) - gstart[g16]
    core = Vs >> 10
    tile = (Vs >> 7) & 7
    part = ((Vs >> 4) & 7) * 16 + (Vs & 15)

    idx = np.full((N_CORES, TILES_PER_CORE, ROWS_PER_TILE, IDXW), PAD_COL, np.uint16)
    idx[core, tile, ((Vs >> 4) & 7) * 16 + (pos & 15), pos >> 4] = Es.astype(np.uint16)
    v8 = np.ones((N_CORES, TILES_PER_CORE, ROWS_PER_TILE, C), np.uint8)
    v8[core, tile, part, pos] = 0
    return order, core, tile, part, pos, idx, v8


def _numpy_fallback(edge_index, logits):
    """Exact reference semantics in numpy (used only if inputs don't fit
    the hardcoded layout, e.g. different shapes/ranges)."""
    V = np.asarray(edge_index[0]).astype(np.int64)
    E = np.asarray(edge_index[1]).astype(np.int64)
    logits = np.asarray(logits)
    raw = logits[V, E]
    scores = (1.0 / (1.0 + np.exp(-raw.astype(np.float64)))).astype(np.float32)
    nnz = V.shape[0]
    order = np.lexsort((-scores, V))
    sorted_v = V[order]
    group_start = np.searchsorted(sorted_v, np.arange(logits.shape[0]))
    rank = np.arange(nnz) - group_start[sorted_v]
    keep_sorted = rank < TOP_K
    keep = np.zeros(nnz, bool)
    keep[order] = keep_sorted
    ei = np.asarray(edge_index)
    pruned = np.where(keep[None, :], ei, ei.dtype.type(-1))
    return pruned, scores


def kernel(edge_index, logits):
    edge_index = np.asarray(edge_index)
    logits = np.asarray(logits)
    V = edge_index[0]
    E = edge_index[1]
    if (edge_index.shape != (2, NNZ) or logits.shape != (NUM_NODES, NUM_EDGES)
            or V.min() < 0 or V.max() >= NUM_USED_NODES
            or E.min() < 0 or E.max() >= NUM_EDGES):
        return _numpy_fallback(edge_index, logits)

    order, core, tile, part, pos, idx, v8 = _pack(edge_index)
    if pos.max() >= C:
        return _numpy_fallback(edge_index, logits)

    logits_f32 = np.ascontiguousarray(logits[:NUM_USED_NODES], dtype=np.float32)
    in_maps = [
        {
            "logits": logits_f32[m * NODES_PER_CORE:(m + 1) * NODES_PER_CORE],
            "idx": idx[m],
            "v8": v8[m],
        }
        for m in range(N_CORES)
    ]
    runner = _build(reps=1)
    res = runner.run(in_maps)

    out_all = np.stack([res[m]["out"] for m in range(N_CORES)])  # [8, T, P, C]
    signed_sorted = out_all[core, tile, part, pos]
    keep_sorted = signed_sorted > 0
    scores_sorted = np.abs(signed_sorted) * np.float32(2.0)

    nnz = edge_index.shape[1]
    scores = np.empty(nnz, np.float32)
    scores[order] = scores_sorted
    keep = np.empty(nnz, bool)
    keep[order] = keep_sorted
    pruned = np.where(keep[None, :], edge_index, edge_index.dtype.type(-1))
    return pruned, scores
